# revision 11
# baseline (speedup 1.0000x reference)
"""LIF readout kernel for Trainium2 (8 NeuronCores, data-parallel over batch).

Reference computation (per element):
    cur[t,b,o] = (x[t] @ W)[b,o] + bias_o + psp          (psp = THRESH/(2T))
    v_t   = DECAY*m_{t-1} + cur_t
    s_t   = (v_t > THRESH)
    m_t   = v_t - s_t*THRESH
    out[b,o] = mean_t s_t

Device strategy per core (B_local = 16 batch rows):
  - GEMM out.T orientation: psum[o_p, (t,b)] = W[:,j*128:+128].T @ xT-block,
    K=2048 via 16 accumulating matmuls into fp32 psum.  x and W are fed as
    float16 (11-bit mantissa; x~N(0,1), W~N(0,1/sqrt(C)) fit comfortably):
    halves DMA traffic, runs 1 cycle/row at any free size, and enables fast
    weight loads.
  - t-chunks sized so every matmul's free dim (16*tb >= 256) exceeds the
    ~117ns LDWEIGHTS shadow -> the PE is purely stream-paced (~0.415ns/row).
    One full j-sweep per chunk (no half splits): psum tag j holds the
    [128, 16*tb] fp32 accumulator, all 8 banks in flight.
  - ~64 warmup matmuls on a zeroed SBUF tile run before the first real
    matmul: they bridge the input-DMA window so the PE is continuously busy
    from ~4us, which ramps the HAM activity governor to full speed (K=8/8)
    before the real GEMM starts (otherwise the first ~20us run at half PE
    clock).
  - DMA order: x chunk 0, W[j0], W[j1..7], then x chunks 1..3; the dge
    spreads each descriptor's pages across 16 DMA engines (~0.4GB/us), so
    ordering (not bandwidth) determines when the first matmul can start.
  - ScalarE copies each psum j-tile into an SBUF currents ring laid out
    [o_p, (t, j, b)] fp16, so one scan step reads a contiguous [128,128]
    slice.
  - VectorE runs the scan as 2 fp16 scalar_tensor_tensor ops per step
    (n = -(m - kappa), kappa = 10*psp, THETA = THRESH - kappa):
        sv      = (sn[(t-1)%3] * -DECAY) + cur[t]   # v_t
        sn[t%3] = (sv is_gt THETA) - sv             # n_t
    while GpSimd accumulates N = sum_t n_t in parallel; sn is
    triple-buffered so GpSimd's trailing read never blocks VectorE.
    Spike counts are recovered at the end from the exact identity
        sum_t s_t = (1-DECAY)*N + DECAY*n_{T-1} - DECAY*kappa + sum_t c_t
    sum_t c_t comes from 16 extra moving columns (host-precomputed
    sum_t x) appended to block XBLK's GEMM.  Membrane quantization at
    2^-11 flips ~0.5% extra near-threshold spikes (rel L2 ~6e-3, well
    inside tolerance).
  - The last t-chunk is the smallest (16): after the GEMM retires, the
    scan only has to sprint 16 trailing steps before the combine.
  - Output: spike counts [o_p=128, (j,b)=128] fp32 DMA'd raw; host
    un-permutes and divides by T.
"""
import numpy as np
from contextlib import ExitStack

import concourse.bass as bass
import concourse.tile as tile
from concourse import bacc, mybir
from concourse.bass_utils import run_bass_kernel_spmd

T, B, C, O = 100, 128, 2048, 1000
NCORES = 8
BL = B // NCORES            # 16 batch rows per core
OP = 1024                   # O padded to 8 o-chunks of 128
NJ = OP // 128              # 8
NK = C // 128               # 16
DECAY = 0.9
THRESH = 1.0
PSP = THRESH / (2 * T)      # 0.005
KAPPA = PSP / (1.0 - DECAY)     # 0.05
THETA = THRESH - KAPPA          # 0.95

# t-steps per pipeline chunk: a scan step's currents are complete only
# when its whole chunk's GEMM retires (all 8 j-sweeps x full K), so the
# chunk size is the scan's handover granularity AND the matmul free dim
# (16*tb).  16 is the sweet spot: free=256 stays ~stream-paced while the
# scan trails the GEMM by only one 16-step chunk.
TBLOCKS = [12, 20, 20, 16, 16, 16]
# xsum columns ride the LAST chunk: they are only needed by the final
# combine, and keeping them out of chunk 0 lands x0 (which gates the
# first matmul) ~0.35us earlier.
XBLK = 5
# Per-j W DMAs split across BOTH HW-DGE rings (even j on the scalar ring,
# odd j on the sync ring right after x0) so each 0.5MB W chunk lands just
# ahead of its first j-sweep at ~2x single-ring rate.  x rides the sync
# ring; chunk 0 is small so x0 clears the ring ASAP.
NWARM = 28                  # warmup matmuls (free=256, one accum group)
assert sum(TBLOCKS) == T
assert all(192 <= 16 * tb <= 512 for tb in TBLOCKS)
assert 16 * TBLOCKS[XBLK] + 16 <= 512

F32 = mybir.dt.float32
F16 = mybir.dt.float16

_cache: dict = {}


def _build(use_bias: bool):
    nc = bacc.Bacc("TRN2", target_bir_lowering=False, debug=False)

    total_cols = NK * (BL * T + 16)   # + xsum columns riding with block XBLK
    x_d = nc.dram_tensor("xp", [128, total_cols], F16, kind="ExternalInput")
    w_d = nc.dram_tensor("wp", [128, NJ * NK * 128], F16, kind="ExternalInput")
    if use_bias:
        b_d = nc.dram_tensor("bp", [1, OP], F16, kind="ExternalInput")
    o_d = nc.dram_tensor("acc_raw", [128, 128], F32, kind="ExternalOutput")

    def sx_of(bi):
        return BL * TBLOCKS[bi] + (16 if bi == XBLK else 0)

    with tile.TileContext(nc) as tc, ExitStack() as ctx:
        wpool = ctx.enter_context(tc.tile_pool(name="wpool", bufs=1))
        xpool = ctx.enter_context(tc.tile_pool(name="xpool", bufs=3))
        cpool = ctx.enter_context(tc.tile_pool(name="cpool", bufs=3))
        spool = ctx.enter_context(tc.tile_pool(name="spool", bufs=1))
        ppool = ctx.enter_context(tc.tile_pool(name="ppool", bufs=1, space="PSUM"))

        # DMA order: first x block, then W (group [j0] unblocks the first
        # j-sweep ASAP), then the remaining x blocks.
        xts = []
        xt0 = xpool.tile([128, NK * sx_of(0)], F16, tag="xt", name="xt0")
        nc.sync.dma_start(xt0[:], x_d[:, 0:NK * sx_of(0)])
        xts.append(xt0)

        # even-j W chunks on the scalar ring (issued immediately, land ~1us
        # apart); odd-j chunks go on the sync ring below, after x0.
        wts = [None] * NJ          # (tile, col offset of j's chunk)
        for j in range(NJ):
            wg = wpool.tile([128, NK * 128], F16, name=f"wg{j}")
            wts[j] = (wg, 0)
        for j in range(0, NJ, 2):
            nc.scalar.dma_start(
                wts[j][0][:], w_d[:, j * NK * 128:(j + 1) * NK * 128])
        for j in range(1, NJ, 2):
            nc.sync.dma_start(
                wts[j][0][:], w_d[:, j * NK * 128:(j + 1) * NK * 128])
        if use_bias:
            bt = wpool.tile([1, OP], F16, name="bt")
            nc.sync.dma_start(bt[:], b_d[:])
            ones = wpool.tile([1, 544], F16, name="ones")
            nc.vector.memset(ones[:], 1.0)
            # the xsum columns need T*b, not b
            oxs = BL * TBLOCKS[XBLK]
            nc.vector.memset(ones[:, oxs:oxs + 16], float(T))

        coff = NK * sx_of(0)
        for bi in range(1, len(TBLOCKS)):
            Sx = sx_of(bi)
            xt = xpool.tile([128, NK * Sx], F16, tag="xt", name=f"xt{bi}")
            nc.sync.dma_start(xt[:], x_d[:, coff:coff + NK * Sx])
            xts.append(xt)
            coff += NK * Sx

        # PE warmup: keep the tensor engine busy through the input-DMA
        # window so HAM ramps to full clock before the real GEMM.  The
        # zeroed operand tile has no DMA dependency; psum tag ps7 is
        # reused (its first real producer is the last j of chunk 0, long
        # after these retire).
        warm = spool.tile([128, 256], F16, name="warm")
        nc.gpsimd.memset(warm[:], 0.0)
        # single accumulation group: back-to-back on the PE queue with no
        # per-matmul semaphores (separate groups would get WAW-serialized
        # by the tile scheduler at ~426ns each)
        wps = ppool.tile([128, 256], F32, tag="ps7", name="warm_ps")
        for i in range(NWARM):
            nc.tensor.matmul(wps[:], warm[:, :128], warm[:],
                             start=(i == 0), stop=(i == NWARM - 1))

        sv = spool.tile([128, 128], F16, name="sv")
        sn = [spool.tile([128, 128], F16, name=f"sn{i}") for i in range(3)]
        nsum = spool.tile([128, 128], F32, name="nsum")
        csum = spool.tile([128, 128], F32, name="csum")
        nc.vector.memset(sn[2][:], KAPPA)   # n_{-1}; first step has t_glob=0
        nc.gpsimd.memset(nsum[:], 0.0)

        t_glob = 0
        for bi, tb in enumerate(TBLOCKS):
            S = BL * tb
            Sx = sx_of(bi)
            xt = xts[bi]
            xtra = 16 if bi == XBLK else 0
            cur = cpool.tile([128, tb * 128], F16, tag="cur", name=f"cur{bi}")
            cur3 = cur[:].rearrange("p (t v) -> p t v", v=128)
            for j in range(NJ):
                ps = ppool.tile([128, S + xtra], F32, tag=f"ps{j}",
                                name=f"ps{bi}_{j}")
                wg, wo = wts[j]
                for k in range(NK):
                    nc.tensor.matmul(
                        ps[:],
                        wg[:, wo + k * 128:wo + (k + 1) * 128],
                        xt[:, k * Sx:k * Sx + S + xtra],
                        start=(k == 0),
                        stop=(k == NK - 1 and not use_bias),
                    )
                if use_bias:
                    nc.tensor.matmul(
                        ps[:],
                        bt[:, j * 128:(j + 1) * 128],
                        ones[:, :S + xtra],
                        start=False,
                        stop=True,
                    )
                # psum [o_p,(t,b)] fp32 -> ring [o_p,(t,j,b)] fp16
                nc.scalar.copy(
                    cur3[:, :tb, j * BL:(j + 1) * BL],
                    ps[:, :S].rearrange("p (t b) -> p t b", b=BL),
                )
                if xtra:
                    nc.scalar.copy(csum[:, j * 16:(j + 1) * 16],
                                   ps[:, S:S + 16])

            # scan: small dedicated state tiles keep each STT near its
            # ~260ns floor.  sn is triple-buffered so GpSimd's read never
            # blocks VectorE's next write.
            for tl in range(tb):
                c_t = cur[:, tl * 128:(tl + 1) * 128]
                n_prev = sn[(t_glob - 1) % 3]
                n_cur = sn[t_glob % 3]
                nc.vector.scalar_tensor_tensor(
                    out=sv[:], in0=n_prev[:], scalar=-DECAY, in1=c_t,
                    op0=mybir.AluOpType.mult, op1=mybir.AluOpType.add)
                nc.vector.scalar_tensor_tensor(
                    out=n_cur[:], in0=sv[:], scalar=THETA, in1=sv[:],
                    op0=mybir.AluOpType.is_gt, op1=mybir.AluOpType.subtract)
                if t_glob != T - 1:
                    # skip the last add: n_{T-1} folds into the combine, so
                    # the tail never waits on GpSimd
                    nc.gpsimd.tensor_tensor(
                        out=nsum[:], in0=nsum[:], in1=n_cur[:],
                        op=mybir.AluOpType.add)
                t_glob += 1

        # combine: out = (1-d)*sum_{t<T-1} n + n_{T-1} + csum
        # (identity: (1-d)*sum_t n + d*n_{T-1} = (1-d)*N' + n_{T-1});
        # the host subtracts the constant d*kappa.
        n_last = sn[(T - 1) % 3]
        nc.vector.scalar_tensor_tensor(
            out=nsum[:], in0=nsum[:], scalar=1.0 - DECAY, in1=csum[:],
            op0=mybir.AluOpType.mult, op1=mybir.AluOpType.add)
        nc.vector.tensor_tensor(
            out=nsum[:], in0=n_last[:], in1=nsum[:],
            op=mybir.AluOpType.add)

        nc.sync.dma_start(o_d[:], nsum[:])

    nc.finalize()
    return nc


def _prep_x(x_core: np.ndarray) -> np.ndarray:
    """x_core [T, BL, C] -> fp16 [128, cols] block-major (k, m) layout, with
    sum_t x appended as 16 extra m-columns per k-slice of block XBLK."""
    xm = np.ascontiguousarray(x_core.reshape(T * BL, C).T)   # [C, M] = [k*128+p, m]
    xk = xm.reshape(NK, 128, T * BL)                         # [k, p, m]
    xs = x_core.sum(axis=0, dtype=np.float64).T.astype(np.float32)  # [C, BL]
    xsk = xs.reshape(NK, 128, BL)                            # [k, p, b]
    segs = []
    m0 = 0
    for bi, tb in enumerate(TBLOCKS):
        S = BL * tb
        seg = xk[:, :, m0:m0 + S]                            # [k, p, S]
        if bi == XBLK:
            seg = np.concatenate([seg, xsk], axis=2)         # [k, p, S+16]
        segs.append(np.ascontiguousarray(seg.transpose(1, 0, 2)).reshape(128, -1))
        m0 += S
    return np.concatenate(segs, axis=1).astype(np.float16)


def kernel(x: np.ndarray, W: np.ndarray, b: np.ndarray) -> np.ndarray:
    x = np.asarray(x, dtype=np.float32)
    W = np.asarray(W, dtype=np.float32)
    b = np.asarray(b, dtype=np.float32)
    use_bias = bool(np.any(b != 0.0))

    key = use_bias
    if key not in _cache:
        _cache[key] = _build(use_bias)
    nc = _cache[key]

    Wp = np.zeros((C, OP), np.float32)
    Wp[:, :O] = W
    # j-major chunks: [128, (j, k, 128)]
    wprep = np.ascontiguousarray(
        Wp.reshape(NK, 128, NJ, 128).transpose(1, 2, 0, 3)
          .reshape(128, NJ * NK * 128)).astype(np.float16)

    in_maps = []
    for c in range(NCORES):
        m = {"xp": _prep_x(x[:, c * BL:(c + 1) * BL, :]), "wp": wprep}
        if use_bias:
            bp = np.zeros((1, OP), np.float16)
            bp[0, :O] = b.astype(np.float16)
            m["bp"] = bp
        in_maps.append(m)

    res = run_bass_kernel_spmd(nc, in_maps, list(range(NCORES)))

    outs = []
    for c in range(NCORES):
        raw = res.results[c]["acc_raw"]                      # [o_p, (j, b)]
        raw = raw - np.float32(DECAY * KAPPA)                # device skips -d*kappa
        rate = raw.reshape(128, NJ, BL).transpose(2, 1, 0).reshape(BL, OP)
        outs.append(rate[:, :O] / np.float32(T))
    return np.concatenate(outs, axis=0).astype(np.float32)


# revision 13
# speedup vs baseline: 1.0421x; 1.0421x over previous
"""LIF readout kernel for Trainium2 (8 NeuronCores, data-parallel over batch).

Reference computation (per element):
    cur[t,b,o] = (x[t] @ W)[b,o] + bias_o + psp          (psp = THRESH/(2T))
    v_t   = DECAY*m_{t-1} + cur_t
    s_t   = (v_t > THRESH)
    m_t   = v_t - s_t*THRESH
    out[b,o] = mean_t s_t

Device strategy per core (B_local = 16 batch rows):
  - GEMM out.T orientation: psum[o_p, (t,b)] = W[:,j*128:+128].T @ xT-block,
    K=2048 via 16 accumulating matmuls into fp32 psum.  x and W are fed as
    float16 (11-bit mantissa; x~N(0,1), W~N(0,1/sqrt(C)) fit comfortably):
    halves DMA traffic, runs 1 cycle/row at any free size, and enables fast
    weight loads.
  - t-chunks sized so every matmul's free dim (16*tb >= 256) exceeds the
    ~117ns LDWEIGHTS shadow -> the PE is purely stream-paced (~0.415ns/row).
    One full j-sweep per chunk (no half splits): psum tag j holds the
    [128, 16*tb] fp32 accumulator, all 8 banks in flight.
  - ~64 warmup matmuls on a zeroed SBUF tile run before the first real
    matmul: they bridge the input-DMA window so the PE is continuously busy
    from ~4us, which ramps the HAM activity governor to full speed (K=8/8)
    before the real GEMM starts (otherwise the first ~20us run at half PE
    clock).
  - DMA order: x chunk 0, W[j0], W[j1..7], then x chunks 1..3; the dge
    spreads each descriptor's pages across 16 DMA engines (~0.4GB/us), so
    ordering (not bandwidth) determines when the first matmul can start.
  - ScalarE copies each psum j-tile into an SBUF currents ring laid out
    [o_p, (t, j, b)] fp16, so one scan step reads a contiguous [128,128]
    slice.
  - VectorE runs the scan as 2 fp16 scalar_tensor_tensor ops per step
    (n = -(m - kappa), kappa = 10*psp, THETA = THRESH - kappa):
        sv      = (sn[(t-1)%3] * -DECAY) + cur[t]   # v_t
        sn[t%3] = (sv is_gt THETA) - sv             # n_t
    while GpSimd accumulates N = sum_t n_t in parallel; sn is
    triple-buffered so GpSimd's trailing read never blocks VectorE.
    Spike counts are recovered at the end from the exact identity
        sum_t s_t = (1-DECAY)*N + DECAY*n_{T-1} - DECAY*kappa + sum_t c_t
    sum_t c_t comes from 16 extra moving columns (host-precomputed
    sum_t x) appended to block XBLK's GEMM.  Membrane quantization at
    2^-11 flips ~0.5% extra near-threshold spikes (rel L2 ~6e-3, well
    inside tolerance).
  - The last t-chunk is the smallest (16): after the GEMM retires, the
    scan only has to sprint 16 trailing steps before the combine.
  - Output: spike counts [o_p=128, (j,b)=128] fp32 DMA'd raw; host
    un-permutes and divides by T.
"""
import numpy as np
from contextlib import ExitStack

import concourse.bass as bass
import concourse.tile as tile
from concourse import bacc, mybir
from concourse.bass_utils import run_bass_kernel_spmd

T, B, C, O = 100, 128, 2048, 1000
NCORES = 8
BL = B // NCORES            # 16 batch rows per core
OP = 1024                   # O padded to 8 o-chunks of 128
NJ = OP // 128              # 8
NK = C // 128               # 16
DECAY = 0.9
THRESH = 1.0
PSP = THRESH / (2 * T)      # 0.005
KAPPA = PSP / (1.0 - DECAY)     # 0.05
THETA = THRESH - KAPPA          # 0.95

# t-steps per pipeline chunk: a scan step's currents are complete only
# when its whole chunk's GEMM retires (all 8 j-sweeps x full K), so the
# chunk size is the scan's handover granularity AND the matmul free dim
# (16*tb).  16 is the sweet spot: free=256 stays ~stream-paced while the
# scan trails the GEMM by only one 16-step chunk.
TBLOCKS = [12, 20, 20, 16, 16, 16]
# xsum columns ride the LAST chunk: they are only needed by the final
# combine, and keeping them out of chunk 0 lands x0 (which gates the
# first matmul) ~0.35us earlier.
XBLK = 5
# Per-j W DMAs split across BOTH HW-DGE rings (even j on the scalar ring,
# odd j on the sync ring right after x0) so each 0.5MB W chunk lands just
# ahead of its first j-sweep at ~2x single-ring rate.  x rides the sync
# ring; chunk 0 is small so x0 clears the ring ASAP.
NWARM = 28                  # warmup matmuls (free=256, one accum group)
assert sum(TBLOCKS) == T
assert all(192 <= 16 * tb <= 512 for tb in TBLOCKS)
assert 16 * TBLOCKS[XBLK] + 16 <= 512

F32 = mybir.dt.float32
F16 = mybir.dt.float16

_cache: dict = {}


def _build(use_bias: bool):
    nc = bacc.Bacc("TRN2", target_bir_lowering=False, debug=False)

    total_cols = NK * (BL * T + 16)   # + xsum columns riding with block XBLK
    x_d = nc.dram_tensor("xp", [128, total_cols], F16, kind="ExternalInput")
    w_d = nc.dram_tensor("wp", [128, NJ * NK * 128], F16, kind="ExternalInput")
    if use_bias:
        b_d = nc.dram_tensor("bp", [1, OP], F16, kind="ExternalInput")
    o_d = nc.dram_tensor("acc_raw", [128, 128], F32, kind="ExternalOutput")

    def sx_of(bi):
        return BL * TBLOCKS[bi] + (16 if bi == XBLK else 0)

    with tile.TileContext(nc) as tc, ExitStack() as ctx:
        wpool = ctx.enter_context(tc.tile_pool(name="wpool", bufs=1))
        xpool = ctx.enter_context(tc.tile_pool(name="xpool", bufs=3))
        cpool = ctx.enter_context(tc.tile_pool(name="cpool", bufs=3))
        spool = ctx.enter_context(tc.tile_pool(name="spool", bufs=1))
        ppool = ctx.enter_context(tc.tile_pool(name="ppool", bufs=1, space="PSUM"))

        # DMA order: first x block, then W (group [j0] unblocks the first
        # j-sweep ASAP), then the remaining x blocks.
        xts = []
        xt0 = xpool.tile([128, NK * sx_of(0)], F16, tag="xt", name="xt0")
        nc.sync.dma_start(xt0[:], x_d[:, 0:NK * sx_of(0)])
        xts.append(xt0)

        # even-j W chunks on the scalar ring (issued immediately, land ~1us
        # apart); odd-j chunks go on the sync ring below, after x0.
        wts = [None] * NJ          # (tile, col offset of j's chunk)
        for j in range(NJ):
            wg = wpool.tile([128, NK * 128], F16, name=f"wg{j}")
            wts[j] = (wg, 0)
        for j in range(0, NJ, 2):
            nc.scalar.dma_start(
                wts[j][0][:], w_d[:, j * NK * 128:(j + 1) * NK * 128])
        for j in range(1, NJ, 2):
            nc.sync.dma_start(
                wts[j][0][:], w_d[:, j * NK * 128:(j + 1) * NK * 128])
        if use_bias:
            bt = wpool.tile([1, OP], F16, name="bt")
            nc.sync.dma_start(bt[:], b_d[:])
            ones = wpool.tile([1, 544], F16, name="ones")
            nc.vector.memset(ones[:], 1.0)
            # the xsum columns need T*b, not b
            oxs = BL * TBLOCKS[XBLK]
            nc.vector.memset(ones[:, oxs:oxs + 16], float(T))

        coff = NK * sx_of(0)
        for bi in range(1, len(TBLOCKS)):
            Sx = sx_of(bi)
            xt = xpool.tile([128, NK * Sx], F16, tag="xt", name=f"xt{bi}")
            nc.sync.dma_start(xt[:], x_d[:, coff:coff + NK * Sx])
            xts.append(xt)
            coff += NK * Sx

        # PE warmup: keep the tensor engine busy through the input-DMA
        # window so HAM ramps to full clock before the real GEMM.  The
        # zeroed operand tile has no DMA dependency; psum tag ps7 is
        # reused (its first real producer is the last j of chunk 0, long
        # after these retire).
        warm = spool.tile([128, 256], F16, name="warm")
        nc.gpsimd.memset(warm[:], 0.0)
        # single accumulation group: back-to-back on the PE queue with no
        # per-matmul semaphores (separate groups would get WAW-serialized
        # by the tile scheduler at ~426ns each)
        wps = ppool.tile([128, 256], F32, tag="ps7", name="warm_ps")
        for i in range(NWARM):
            nc.tensor.matmul(wps[:], warm[:, :128], warm[:],
                             start=(i == 0), stop=(i == NWARM - 1))

        # sv is ping-ponged: the scheduler elides adjacent same-engine
        # hazards but inserts a ~220ns completion-wait for the distance-2
        # WAW on a single sv buffer; alternating buffers pushes that
        # dependency out to distance 4 where the semaphore is long posted.
        sv = [spool.tile([128, 128], F16, name=f"sv{i}") for i in range(2)]
        sn = [spool.tile([128, 128], F16, name=f"sn{i}") for i in range(3)]
        nsum = spool.tile([128, 128], F32, name="nsum")
        csum = spool.tile([128, 128], F32, name="csum")
        nc.vector.memset(sn[2][:], KAPPA)   # n_{-1}; first step has t_glob=0
        nc.gpsimd.memset(nsum[:], 0.0)

        t_glob = 0
        for bi, tb in enumerate(TBLOCKS):
            S = BL * tb
            Sx = sx_of(bi)
            xt = xts[bi]
            xtra = 16 if bi == XBLK else 0
            cur = cpool.tile([128, tb * 128], F16, tag="cur", name=f"cur{bi}")
            cur3 = cur[:].rearrange("p (t v) -> p t v", v=128)
            for j in range(NJ):
                ps = ppool.tile([128, S + xtra], F32, tag=f"ps{j}",
                                name=f"ps{bi}_{j}")
                wg, wo = wts[j]
                for k in range(NK):
                    nc.tensor.matmul(
                        ps[:],
                        wg[:, wo + k * 128:wo + (k + 1) * 128],
                        xt[:, k * Sx:k * Sx + S + xtra],
                        start=(k == 0),
                        stop=(k == NK - 1 and not use_bias),
                    )
                if use_bias:
                    nc.tensor.matmul(
                        ps[:],
                        bt[:, j * 128:(j + 1) * 128],
                        ones[:, :S + xtra],
                        start=False,
                        stop=True,
                    )
                # psum [o_p,(t,b)] fp32 -> ring [o_p,(t,j,b)] fp16
                nc.scalar.copy(
                    cur3[:, :tb, j * BL:(j + 1) * BL],
                    ps[:, :S].rearrange("p (t b) -> p t b", b=BL),
                )
                if xtra:
                    nc.scalar.copy(csum[:, j * 16:(j + 1) * 16],
                                   ps[:, S:S + 16])

            # scan: small dedicated state tiles keep each STT near its
            # ~260ns floor.  sn is triple-buffered so GpSimd's read never
            # blocks VectorE's next write.
            for tl in range(tb):
                c_t = cur[:, tl * 128:(tl + 1) * 128]
                n_prev = sn[(t_glob - 1) % 3]
                n_cur = sn[t_glob % 3]
                svt = sv[t_glob % 2]
                nc.vector.scalar_tensor_tensor(
                    out=svt[:], in0=n_prev[:], scalar=-DECAY, in1=c_t,
                    op0=mybir.AluOpType.mult, op1=mybir.AluOpType.add)
                nc.vector.scalar_tensor_tensor(
                    out=n_cur[:], in0=svt[:], scalar=THETA, in1=svt[:],
                    op0=mybir.AluOpType.is_gt, op1=mybir.AluOpType.subtract)
                if t_glob != T - 1:
                    # skip the last add: n_{T-1} folds into the combine, so
                    # the tail never waits on GpSimd
                    nc.gpsimd.tensor_tensor(
                        out=nsum[:], in0=nsum[:], in1=n_cur[:],
                        op=mybir.AluOpType.add)
                t_glob += 1

        # combine: out = (1-d)*sum_{t<T-1} n + n_{T-1} + csum
        # (identity: (1-d)*sum_t n + d*n_{T-1} = (1-d)*N' + n_{T-1});
        # the host subtracts the constant d*kappa.
        n_last = sn[(T - 1) % 3]
        nc.vector.scalar_tensor_tensor(
            out=nsum[:], in0=nsum[:], scalar=1.0 - DECAY, in1=csum[:],
            op0=mybir.AluOpType.mult, op1=mybir.AluOpType.add)
        nc.vector.tensor_tensor(
            out=nsum[:], in0=n_last[:], in1=nsum[:],
            op=mybir.AluOpType.add)

        nc.sync.dma_start(o_d[:], nsum[:])

    nc.finalize()
    return nc


def _prep_x(x_core: np.ndarray) -> np.ndarray:
    """x_core [T, BL, C] -> fp16 [128, cols] block-major (k, m) layout, with
    sum_t x appended as 16 extra m-columns per k-slice of block XBLK."""
    xm = np.ascontiguousarray(x_core.reshape(T * BL, C).T)   # [C, M] = [k*128+p, m]
    xk = xm.reshape(NK, 128, T * BL)                         # [k, p, m]
    xs = x_core.sum(axis=0, dtype=np.float64).T.astype(np.float32)  # [C, BL]
    xsk = xs.reshape(NK, 128, BL)                            # [k, p, b]
    segs = []
    m0 = 0
    for bi, tb in enumerate(TBLOCKS):
        S = BL * tb
        seg = xk[:, :, m0:m0 + S]                            # [k, p, S]
        if bi == XBLK:
            seg = np.concatenate([seg, xsk], axis=2)         # [k, p, S+16]
        segs.append(np.ascontiguousarray(seg.transpose(1, 0, 2)).reshape(128, -1))
        m0 += S
    return np.concatenate(segs, axis=1).astype(np.float16)


def kernel(x: np.ndarray, W: np.ndarray, b: np.ndarray) -> np.ndarray:
    x = np.asarray(x, dtype=np.float32)
    W = np.asarray(W, dtype=np.float32)
    b = np.asarray(b, dtype=np.float32)
    use_bias = bool(np.any(b != 0.0))

    key = use_bias
    if key not in _cache:
        _cache[key] = _build(use_bias)
    nc = _cache[key]

    Wp = np.zeros((C, OP), np.float32)
    Wp[:, :O] = W
    # j-major chunks: [128, (j, k, 128)]
    wprep = np.ascontiguousarray(
        Wp.reshape(NK, 128, NJ, 128).transpose(1, 2, 0, 3)
          .reshape(128, NJ * NK * 128)).astype(np.float16)

    in_maps = []
    for c in range(NCORES):
        m = {"xp": _prep_x(x[:, c * BL:(c + 1) * BL, :]), "wp": wprep}
        if use_bias:
            bp = np.zeros((1, OP), np.float16)
            bp[0, :O] = b.astype(np.float16)
            m["bp"] = bp
        in_maps.append(m)

    res = run_bass_kernel_spmd(nc, in_maps, list(range(NCORES)))

    outs = []
    for c in range(NCORES):
        raw = res.results[c]["acc_raw"]                      # [o_p, (j, b)]
        raw = raw - np.float32(DECAY * KAPPA)                # device skips -d*kappa
        rate = raw.reshape(128, NJ, BL).transpose(2, 1, 0).reshape(BL, OP)
        outs.append(rate[:, :O] / np.float32(T))
    return np.concatenate(outs, axis=0).astype(np.float32)


# revision 16
# speedup vs baseline: 1.0475x; 1.0051x over previous
"""LIF readout kernel for Trainium2 (8 NeuronCores, data-parallel over batch).

Reference computation (per element):
    cur[t,b,o] = (x[t] @ W)[b,o] + bias_o + psp          (psp = THRESH/(2T))
    v_t   = DECAY*m_{t-1} + cur_t
    s_t   = (v_t > THRESH)
    m_t   = v_t - s_t*THRESH
    out[b,o] = mean_t s_t

Device strategy per core (B_local = 16 batch rows):
  - GEMM out.T orientation: psum[o_p, (t,b)] = W[:,j*128:+128].T @ xT-block,
    K=2048 via 16 accumulating matmuls into fp32 psum.  x and W are fed as
    float16 (11-bit mantissa; x~N(0,1), W~N(0,1/sqrt(C)) fit comfortably):
    halves DMA traffic, runs 1 cycle/row at any free size, and enables fast
    weight loads.
  - A scan step's currents are complete only when its whole chunk's GEMM
    retires (all 8 j-sweeps x full K), so chunk size is both the scan's
    handover granularity and the matmul free dim (16*tb).  Chunks of
    12-20 steps keep the PE stream-paced (~0.415ns/row; LDWEIGHTS hides
    under the >=192-col stream) while the scan trails the GEMM by only
    one chunk; the last chunk (16) is the post-GEMM scan sprint.
  - Warmup matmuls on a zeroed SBUF tile (one accumulation group -> no
    per-matmul semaphores) bridge the input-DMA window so the PE is
    continuously busy from ~7us, ramping the HAM activity governor to
    full clock (K=8/8) before the real GEMM starts (a cold or re-throttled
    PE runs at half clock for several us).
  - Input DMA uses BOTH HW-DGE rings: x chunks stream on the sync ring
    (x0 first and small, so the first j-sweep can start ~13us in), the
    eight 0.5MB per-j W chunks alternate scalar ring (even j, issued
    first) / sync ring (odd j, after x0), landing each W_j just ahead of
    chunk 0's j-sweep.  Startup is per-packet-overhead-bound (~128 row
    packets per descriptor), not bandwidth-bound.
  - ScalarE copies each psum j-tile into an SBUF currents ring laid out
    [o_p, (t, j, b)] fp16, so one scan step reads a contiguous [128,128]
    slice.
  - VectorE runs the scan as 2 fp16 scalar_tensor_tensor ops per step
    (n = -(m - kappa), kappa = 10*psp, THETA = THRESH - kappa):
        sv      = (sn[(t-1)%3] * -DECAY) + cur[t]   # v_t
        sn[t%3] = (sv is_gt THETA) - sv             # n_t
    while GpSimd accumulates N = sum_t n_t in parallel; sn is
    triple-buffered so GpSimd's trailing read never blocks VectorE.
    Spike counts are recovered at the end from the exact identity
        sum_t s_t = (1-DECAY)*N + DECAY*n_{T-1} - DECAY*kappa + sum_t c_t
    sum_t c_t comes from 16 extra moving columns (host-precomputed
    sum_t x) appended to block XBLK's GEMM (the LAST chunk: csum is only
    needed by the combine, and this keeps x0 minimal).  Membrane
    quantization at 2^-11 flips ~0.5% extra near-threshold spikes
    (rel L2 ~5e-3, well inside tolerance).
  - Output: spike counts [o_p=128, (j,b)=128] fp32 DMA'd raw; host
    un-permutes and divides by T.
"""
import numpy as np
from contextlib import ExitStack

import concourse.bass as bass
import concourse.tile as tile
from concourse import bacc, mybir
from concourse.bass_utils import run_bass_kernel_spmd

T, B, C, O = 100, 128, 2048, 1000
NCORES = 8
BL = B // NCORES            # 16 batch rows per core
OP = 1024                   # O padded to 8 o-chunks of 128
NJ = OP // 128              # 8
NK = C // 128               # 16
DECAY = 0.9
THRESH = 1.0
PSP = THRESH / (2 * T)      # 0.005
KAPPA = PSP / (1.0 - DECAY)     # 0.05
THETA = THRESH - KAPPA          # 0.95

# t-steps per pipeline chunk: a scan step's currents are complete only
# when its whole chunk's GEMM retires (all 8 j-sweeps x full K), so the
# chunk size is the scan's handover granularity AND the matmul free dim
# (16*tb).  16 is the sweet spot: free=256 stays ~stream-paced while the
# scan trails the GEMM by only one 16-step chunk.
TBLOCKS = [12, 20, 20, 18, 18, 12]
# xsum columns ride the LAST chunk: they are only needed by the final
# combine, and keeping them out of chunk 0 lands x0 (which gates the
# first matmul) ~0.35us earlier.
XBLK = 5
# Per-j W DMAs split across BOTH HW-DGE rings (even j on the scalar ring,
# odd j on the sync ring right after x0) so each 0.5MB W chunk lands just
# ahead of its first j-sweep at ~2x single-ring rate.  x rides the sync
# ring; chunk 0 is small so x0 clears the ring ASAP.
NWARM = 28                  # warmup matmuls (free=256, one accum group)
assert sum(TBLOCKS) == T
assert all(192 <= 16 * tb <= 512 for tb in TBLOCKS)
assert 16 * TBLOCKS[XBLK] + 16 <= 512

F32 = mybir.dt.float32
F16 = mybir.dt.float16

_cache: dict = {}


def _build(use_bias: bool):
    nc = bacc.Bacc("TRN2", target_bir_lowering=False, debug=False)

    total_cols = NK * (BL * T + 16)   # + xsum columns riding with block XBLK
    x_d = nc.dram_tensor("xp", [128, total_cols], F16, kind="ExternalInput")
    w_d = nc.dram_tensor("wp", [128, NJ * NK * 128], F16, kind="ExternalInput")
    if use_bias:
        b_d = nc.dram_tensor("bp", [1, OP], F16, kind="ExternalInput")
    o_d = nc.dram_tensor("acc_raw", [128, 128], F32, kind="ExternalOutput")

    def sx_of(bi):
        return BL * TBLOCKS[bi] + (16 if bi == XBLK else 0)

    with tile.TileContext(nc) as tc, ExitStack() as ctx:
        wpool = ctx.enter_context(tc.tile_pool(name="wpool", bufs=1))
        xpool = ctx.enter_context(tc.tile_pool(name="xpool", bufs=3))
        cpool = ctx.enter_context(tc.tile_pool(name="cpool", bufs=3))
        spool = ctx.enter_context(tc.tile_pool(name="spool", bufs=1))
        ppool = ctx.enter_context(tc.tile_pool(name="ppool", bufs=1, space="PSUM"))

        # DMA order: first x block, then W (group [j0] unblocks the first
        # j-sweep ASAP), then the remaining x blocks.
        xts = []
        xt0 = xpool.tile([128, NK * sx_of(0)], F16, tag="xt", name="xt0")
        nc.sync.dma_start(xt0[:], x_d[:, 0:NK * sx_of(0)])
        xts.append(xt0)

        # even-j W chunks on the scalar ring (issued immediately, land ~1us
        # apart); odd-j chunks go on the sync ring below, after x0.
        wts = [None] * NJ          # (tile, col offset of j's chunk)
        for j in range(NJ):
            wg = wpool.tile([128, NK * 128], F16, name=f"wg{j}")
            wts[j] = (wg, 0)
        for j in range(0, NJ, 2):
            nc.scalar.dma_start(
                wts[j][0][:], w_d[:, j * NK * 128:(j + 1) * NK * 128])
        for j in range(1, NJ, 2):
            nc.sync.dma_start(
                wts[j][0][:], w_d[:, j * NK * 128:(j + 1) * NK * 128])
        if use_bias:
            bt = wpool.tile([1, OP], F16, name="bt")
            nc.sync.dma_start(bt[:], b_d[:])
            ones = wpool.tile([1, 544], F16, name="ones")
            nc.vector.memset(ones[:], 1.0)
            # the xsum columns need T*b, not b
            oxs = BL * TBLOCKS[XBLK]
            nc.vector.memset(ones[:, oxs:oxs + 16], float(T))

        coff = NK * sx_of(0)
        for bi in range(1, len(TBLOCKS)):
            Sx = sx_of(bi)
            xt = xpool.tile([128, NK * Sx], F16, tag="xt", name=f"xt{bi}")
            nc.sync.dma_start(xt[:], x_d[:, coff:coff + NK * Sx])
            xts.append(xt)
            coff += NK * Sx

        # PE warmup: keep the tensor engine busy through the input-DMA
        # window so HAM ramps to full clock before the real GEMM.  The
        # zeroed operand tile has no DMA dependency; psum tag ps7 is
        # reused (its first real producer is the last j of chunk 0, long
        # after these retire).
        warm = spool.tile([128, 256], F16, name="warm")
        nc.gpsimd.memset(warm[:], 0.0)
        # single accumulation group: back-to-back on the PE queue with no
        # per-matmul semaphores (separate groups would get WAW-serialized
        # by the tile scheduler at ~426ns each)
        wps = ppool.tile([128, 256], F32, tag="ps7", name="warm_ps")
        for i in range(NWARM):
            nc.tensor.matmul(wps[:], warm[:, :128], warm[:],
                             start=(i == 0), stop=(i == NWARM - 1))

        # sv is ping-ponged: the scheduler elides adjacent same-engine
        # hazards but inserts a ~220ns completion-wait for the distance-2
        # WAW on a single sv buffer; alternating buffers pushes that
        # dependency out to distance 4 where the semaphore is long posted.
        sv = [spool.tile([128, 128], F16, name=f"sv{i}") for i in range(2)]
        sn = [spool.tile([128, 128], F16, name=f"sn{i}") for i in range(3)]
        nsum = spool.tile([128, 128], F32, name="nsum")
        csum = spool.tile([128, 128], F32, name="csum")
        nc.vector.memset(sn[2][:], KAPPA)   # n_{-1}; first step has t_glob=0
        nc.gpsimd.memset(nsum[:], 0.0)

        t_glob = 0
        for bi, tb in enumerate(TBLOCKS):
            S = BL * tb
            Sx = sx_of(bi)
            xt = xts[bi]
            xtra = 16 if bi == XBLK else 0
            cur = cpool.tile([128, tb * 128], F16, tag="cur", name=f"cur{bi}")
            cur3 = cur[:].rearrange("p (t v) -> p t v", v=128)
            for j in range(NJ):
                ps = ppool.tile([128, S + xtra], F32, tag=f"ps{j}",
                                name=f"ps{bi}_{j}")
                wg, wo = wts[j]
                for k in range(NK):
                    nc.tensor.matmul(
                        ps[:],
                        wg[:, wo + k * 128:wo + (k + 1) * 128],
                        xt[:, k * Sx:k * Sx + S + xtra],
                        start=(k == 0),
                        stop=(k == NK - 1 and not use_bias),
                    )
                if use_bias:
                    nc.tensor.matmul(
                        ps[:],
                        bt[:, j * 128:(j + 1) * 128],
                        ones[:, :S + xtra],
                        start=False,
                        stop=True,
                    )
                # psum [o_p,(t,b)] fp32 -> ring [o_p,(t,j,b)] fp16
                nc.scalar.copy(
                    cur3[:, :tb, j * BL:(j + 1) * BL],
                    ps[:, :S].rearrange("p (t b) -> p t b", b=BL),
                )
                if xtra:
                    nc.scalar.copy(csum[:, j * 16:(j + 1) * 16],
                                   ps[:, S:S + 16])

            # scan: small dedicated state tiles keep each STT near its
            # ~260ns floor.  sn is triple-buffered so GpSimd's read never
            # blocks VectorE's next write.
            for tl in range(tb):
                c_t = cur[:, tl * 128:(tl + 1) * 128]
                n_prev = sn[(t_glob - 1) % 3]
                n_cur = sn[t_glob % 3]
                svt = sv[t_glob % 2]
                nc.vector.scalar_tensor_tensor(
                    out=svt[:], in0=n_prev[:], scalar=-DECAY, in1=c_t,
                    op0=mybir.AluOpType.mult, op1=mybir.AluOpType.add)
                nc.vector.scalar_tensor_tensor(
                    out=n_cur[:], in0=svt[:], scalar=THETA, in1=svt[:],
                    op0=mybir.AluOpType.is_gt, op1=mybir.AluOpType.subtract)
                if t_glob != T - 1:
                    # skip the last add: n_{T-1} folds into the combine, so
                    # the tail never waits on GpSimd
                    nc.gpsimd.tensor_tensor(
                        out=nsum[:], in0=nsum[:], in1=n_cur[:],
                        op=mybir.AluOpType.add)
                t_glob += 1

        # combine: out = (1-d)*sum_{t<T-1} n + n_{T-1} + csum
        # (identity: (1-d)*sum_t n + d*n_{T-1} = (1-d)*N' + n_{T-1});
        # the host subtracts the constant d*kappa.
        n_last = sn[(T - 1) % 3]
        nc.vector.scalar_tensor_tensor(
            out=nsum[:], in0=nsum[:], scalar=1.0 - DECAY, in1=csum[:],
            op0=mybir.AluOpType.mult, op1=mybir.AluOpType.add)
        nc.vector.tensor_tensor(
            out=nsum[:], in0=n_last[:], in1=nsum[:],
            op=mybir.AluOpType.add)

        nc.sync.dma_start(o_d[:], nsum[:])

    nc.finalize()
    return nc


def _prep_x(x_core: np.ndarray) -> np.ndarray:
    """x_core [T, BL, C] -> fp16 [128, cols] block-major (k, m) layout, with
    sum_t x appended as 16 extra m-columns per k-slice of block XBLK."""
    xm = np.ascontiguousarray(x_core.reshape(T * BL, C).T)   # [C, M] = [k*128+p, m]
    xk = xm.reshape(NK, 128, T * BL)                         # [k, p, m]
    xs = x_core.sum(axis=0, dtype=np.float64).T.astype(np.float32)  # [C, BL]
    xsk = xs.reshape(NK, 128, BL)                            # [k, p, b]
    segs = []
    m0 = 0
    for bi, tb in enumerate(TBLOCKS):
        S = BL * tb
        seg = xk[:, :, m0:m0 + S]                            # [k, p, S]
        if bi == XBLK:
            seg = np.concatenate([seg, xsk], axis=2)         # [k, p, S+16]
        segs.append(np.ascontiguousarray(seg.transpose(1, 0, 2)).reshape(128, -1))
        m0 += S
    return np.concatenate(segs, axis=1).astype(np.float16)


def kernel(x: np.ndarray, W: np.ndarray, b: np.ndarray) -> np.ndarray:
    x = np.asarray(x, dtype=np.float32)
    W = np.asarray(W, dtype=np.float32)
    b = np.asarray(b, dtype=np.float32)
    use_bias = bool(np.any(b != 0.0))

    key = use_bias
    if key not in _cache:
        _cache[key] = _build(use_bias)
    nc = _cache[key]

    Wp = np.zeros((C, OP), np.float32)
    Wp[:, :O] = W
    # j-major chunks: [128, (j, k, 128)]
    wprep = np.ascontiguousarray(
        Wp.reshape(NK, 128, NJ, 128).transpose(1, 2, 0, 3)
          .reshape(128, NJ * NK * 128)).astype(np.float16)

    in_maps = []
    for c in range(NCORES):
        m = {"xp": _prep_x(x[:, c * BL:(c + 1) * BL, :]), "wp": wprep}
        if use_bias:
            bp = np.zeros((1, OP), np.float16)
            bp[0, :O] = b.astype(np.float16)
            m["bp"] = bp
        in_maps.append(m)

    res = run_bass_kernel_spmd(nc, in_maps, list(range(NCORES)))

    outs = []
    for c in range(NCORES):
        raw = res.results[c]["acc_raw"]                      # [o_p, (j, b)]
        raw = raw - np.float32(DECAY * KAPPA)                # device skips -d*kappa
        rate = raw.reshape(128, NJ, BL).transpose(2, 1, 0).reshape(BL, OP)
        outs.append(rate[:, :O] / np.float32(T))
    return np.concatenate(outs, axis=0).astype(np.float32)


# revision 17
# speedup vs baseline: 1.0687x; 1.0203x over previous
"""LIF readout kernel for Trainium2 (8 NeuronCores, data-parallel over batch).

Reference computation (per element):
    cur[t,b,o] = (x[t] @ W)[b,o] + bias_o + psp          (psp = THRESH/(2T))
    v_t   = DECAY*m_{t-1} + cur_t
    s_t   = (v_t > THRESH)
    m_t   = v_t - s_t*THRESH
    out[b,o] = mean_t s_t

Device strategy per core (B_local = 16 batch rows):
  - GEMM out.T orientation: psum[o_p, (t,b)] = W[:,j*128:+128].T @ xT-block,
    K=2048 via 16 accumulating matmuls into fp32 psum.  x and W are fed as
    float16 (11-bit mantissa; x~N(0,1), W~N(0,1/sqrt(C)) fit comfortably):
    halves DMA traffic, runs 1 cycle/row at any free size, and enables fast
    weight loads.
  - A scan step's currents are complete only when its whole chunk's GEMM
    retires (all 8 j-sweeps x full K), so chunk size is both the scan's
    handover granularity and the matmul free dim (16*tb).  Chunks of
    12-20 steps keep the PE stream-paced (~0.415ns/row; LDWEIGHTS hides
    under the >=192-col stream) while the scan trails the GEMM by only
    one chunk; the last chunk (16) is the post-GEMM scan sprint.
  - Warmup matmuls on a zeroed SBUF tile (one accumulation group -> no
    per-matmul semaphores) bridge the input-DMA window so the PE is
    continuously busy from ~7us, ramping the HAM activity governor to
    full clock (K=8/8) before the real GEMM starts (a cold or re-throttled
    PE runs at half clock for several us).
  - Input DMA uses BOTH HW-DGE rings: x chunks stream on the sync ring
    (x0 first and small, so the first j-sweep can start ~13us in), the
    eight 0.5MB per-j W chunks alternate scalar ring (even j, issued
    first) / sync ring (odd j, after x0), landing each W_j just ahead of
    chunk 0's j-sweep.  Startup is per-packet-overhead-bound (~128 row
    packets per descriptor), not bandwidth-bound.
  - ScalarE copies each psum j-tile into an SBUF currents ring laid out
    [o_p, (t, j, b)] fp16, so one scan step reads a contiguous [128,128]
    slice.
  - VectorE runs the scan as 2 fp16 scalar_tensor_tensor ops per step
    (n = -(m - kappa), kappa = 10*psp, THETA = THRESH - kappa):
        sv      = (sn[(t-1)%3] * -DECAY) + cur[t]   # v_t
        sn[t%3] = (sv is_gt THETA) - sv             # n_t
    while GpSimd accumulates N = sum_t n_t in parallel; sn is
    triple-buffered so GpSimd's trailing read never blocks VectorE.
    Spike counts are recovered at the end from the exact identity
        sum_t s_t = (1-DECAY)*N + DECAY*n_{T-1} - DECAY*kappa + sum_t c_t
    sum_t c_t comes from 16 extra moving columns (host-precomputed
    sum_t x) appended to block XBLK's GEMM (the LAST chunk: csum is only
    needed by the combine, and this keeps x0 minimal).  Membrane
    quantization at 2^-11 flips ~0.5% extra near-threshold spikes
    (rel L2 ~5e-3, well inside tolerance).
  - Output: spike counts [o_p=128, (j,b)=128] fp32 DMA'd raw; host
    un-permutes and divides by T.
"""
import numpy as np
from contextlib import ExitStack

import concourse.bass as bass
import concourse.tile as tile
from concourse import bacc, mybir
from concourse.bass_utils import run_bass_kernel_spmd

T, B, C, O = 100, 128, 2048, 1000
NCORES = 8
BL = B // NCORES            # 16 batch rows per core
OP = 1024                   # O padded to 8 o-chunks of 128
NJ = OP // 128              # 8
NK = C // 128               # 16
DECAY = 0.9
THRESH = 1.0
PSP = THRESH / (2 * T)      # 0.005
KAPPA = PSP / (1.0 - DECAY)     # 0.05
THETA = THRESH - KAPPA          # 0.95

# t-steps per pipeline chunk: a scan step's currents are complete only
# when its whole chunk's GEMM retires (all 8 j-sweeps x full K), so the
# chunk size is the scan's handover granularity AND the matmul free dim
# (16*tb).  16 is the sweet spot: free=256 stays ~stream-paced while the
# scan trails the GEMM by only one 16-step chunk.
TBLOCKS = [14, 20, 18, 18, 16, 14]
# xsum columns ride the LAST chunk: they are only needed by the final
# combine, and keeping them out of chunk 0 lands x0 (which gates the
# first matmul) ~0.35us earlier.
XBLK = 5
# Per-j W DMAs split across BOTH HW-DGE rings (even j on the scalar ring,
# odd j on the sync ring right after x0) so each 0.5MB W chunk lands just
# ahead of its first j-sweep at ~2x single-ring rate.  x rides the sync
# ring; chunk 0 is small so x0 clears the ring ASAP.
NWARM = 28                  # warmup matmuls (free=256, one accum group)
assert sum(TBLOCKS) == T
assert all(192 <= 16 * tb <= 512 for tb in TBLOCKS)
assert 16 * TBLOCKS[XBLK] + 16 <= 512

F32 = mybir.dt.float32
F16 = mybir.dt.float16

_cache: dict = {}


def _build(use_bias: bool):
    nc = bacc.Bacc("TRN2", target_bir_lowering=False, debug=False)

    total_cols = NK * (BL * T + 16)   # + xsum columns riding with block XBLK
    x_d = nc.dram_tensor("xp", [128, total_cols], F16, kind="ExternalInput")
    w_d = nc.dram_tensor("wp", [128, NJ * NK * 128], F16, kind="ExternalInput")
    if use_bias:
        b_d = nc.dram_tensor("bp", [1, OP], F16, kind="ExternalInput")
    o_d = nc.dram_tensor("acc_raw", [128, 128], F32, kind="ExternalOutput")

    def sx_of(bi):
        return BL * TBLOCKS[bi] + (16 if bi == XBLK else 0)

    with tile.TileContext(nc) as tc, ExitStack() as ctx:
        wpool = ctx.enter_context(tc.tile_pool(name="wpool", bufs=1))
        xpool = ctx.enter_context(tc.tile_pool(name="xpool", bufs=3))
        cpool = ctx.enter_context(tc.tile_pool(name="cpool", bufs=3))
        spool = ctx.enter_context(tc.tile_pool(name="spool", bufs=1))
        ppool = ctx.enter_context(tc.tile_pool(name="ppool", bufs=1, space="PSUM"))

        # DMA order: first x block, then W (group [j0] unblocks the first
        # j-sweep ASAP), then the remaining x blocks.
        xts = []
        xt0 = xpool.tile([128, NK * sx_of(0)], F16, tag="xt", name="xt0")
        nc.sync.dma_start(xt0[:], x_d[:, 0:NK * sx_of(0)])
        xts.append(xt0)

        # even-j W chunks on the scalar ring (issued immediately, land ~1us
        # apart); odd-j chunks go on the sync ring below, after x0.
        wts = [None] * NJ          # (tile, col offset of j's chunk)
        for j in range(NJ):
            wg = wpool.tile([128, NK * 128], F16, name=f"wg{j}")
            wts[j] = (wg, 0)
        for j in range(0, NJ, 2):
            nc.scalar.dma_start(
                wts[j][0][:], w_d[:, j * NK * 128:(j + 1) * NK * 128])
        for j in range(1, NJ, 2):
            nc.sync.dma_start(
                wts[j][0][:], w_d[:, j * NK * 128:(j + 1) * NK * 128])
        if use_bias:
            bt = wpool.tile([1, OP], F16, name="bt")
            nc.sync.dma_start(bt[:], b_d[:])
            ones = wpool.tile([1, 544], F16, name="ones")
            nc.vector.memset(ones[:], 1.0)
            # the xsum columns need T*b, not b
            oxs = BL * TBLOCKS[XBLK]
            nc.vector.memset(ones[:, oxs:oxs + 16], float(T))

        coff = NK * sx_of(0)
        for bi in range(1, len(TBLOCKS)):
            Sx = sx_of(bi)
            xt = xpool.tile([128, NK * Sx], F16, tag="xt", name=f"xt{bi}")
            nc.sync.dma_start(xt[:], x_d[:, coff:coff + NK * Sx])
            xts.append(xt)
            coff += NK * Sx

        # PE warmup: keep the tensor engine busy through the input-DMA
        # window so HAM ramps to full clock before the real GEMM.  The
        # zeroed operand tile has no DMA dependency; psum tag ps7 is
        # reused (its first real producer is the last j of chunk 0, long
        # after these retire).
        warm = spool.tile([128, 256], F16, name="warm")
        nc.gpsimd.memset(warm[:], 0.0)
        # single accumulation group: back-to-back on the PE queue with no
        # per-matmul semaphores (separate groups would get WAW-serialized
        # by the tile scheduler at ~426ns each)
        wps = ppool.tile([128, 256], F32, tag="ps7", name="warm_ps")
        for i in range(NWARM):
            nc.tensor.matmul(wps[:], warm[:, :128], warm[:],
                             start=(i == 0), stop=(i == NWARM - 1))

        # sv is ping-ponged: the scheduler elides adjacent same-engine
        # hazards but inserts a ~220ns completion-wait for the distance-2
        # WAW on a single sv buffer; alternating buffers pushes that
        # dependency out to distance 4 where the semaphore is long posted.
        sv = [spool.tile([128, 128], F16, name=f"sv{i}") for i in range(2)]
        sn = [spool.tile([128, 128], F16, name=f"sn{i}") for i in range(3)]
        nsum = spool.tile([128, 128], F32, name="nsum")
        csum = spool.tile([128, 128], F32, name="csum")
        nc.vector.memset(sn[2][:], KAPPA)   # n_{-1}; first step has t_glob=0
        nc.gpsimd.memset(nsum[:], 0.0)

        t_glob = 0
        for bi, tb in enumerate(TBLOCKS):
            S = BL * tb
            Sx = sx_of(bi)
            xt = xts[bi]
            xtra = 16 if bi == XBLK else 0
            cur = cpool.tile([128, tb * 128], F16, tag="cur", name=f"cur{bi}")
            cur3 = cur[:].rearrange("p (t v) -> p t v", v=128)
            for j in range(NJ):
                ps = ppool.tile([128, S + xtra], F32, tag=f"ps{j}",
                                name=f"ps{bi}_{j}")
                wg, wo = wts[j]
                for k in range(NK):
                    nc.tensor.matmul(
                        ps[:],
                        wg[:, wo + k * 128:wo + (k + 1) * 128],
                        xt[:, k * Sx:k * Sx + S + xtra],
                        start=(k == 0),
                        stop=(k == NK - 1 and not use_bias),
                    )
                if use_bias:
                    nc.tensor.matmul(
                        ps[:],
                        bt[:, j * 128:(j + 1) * 128],
                        ones[:, :S + xtra],
                        start=False,
                        stop=True,
                    )
                # psum [o_p,(t,b)] fp32 -> ring [o_p,(t,j,b)] fp16
                nc.scalar.copy(
                    cur3[:, :tb, j * BL:(j + 1) * BL],
                    ps[:, :S].rearrange("p (t b) -> p t b", b=BL),
                )
                if xtra:
                    nc.scalar.copy(csum[:, j * 16:(j + 1) * 16],
                                   ps[:, S:S + 16])

            # scan: small dedicated state tiles keep each STT near its
            # ~260ns floor.  sn is triple-buffered so GpSimd's read never
            # blocks VectorE's next write.
            for tl in range(tb):
                c_t = cur[:, tl * 128:(tl + 1) * 128]
                n_prev = sn[(t_glob - 1) % 3]
                n_cur = sn[t_glob % 3]
                svt = sv[t_glob % 2]
                nc.vector.scalar_tensor_tensor(
                    out=svt[:], in0=n_prev[:], scalar=-DECAY, in1=c_t,
                    op0=mybir.AluOpType.mult, op1=mybir.AluOpType.add)
                nc.vector.scalar_tensor_tensor(
                    out=n_cur[:], in0=svt[:], scalar=THETA, in1=svt[:],
                    op0=mybir.AluOpType.is_gt, op1=mybir.AluOpType.subtract)
                if t_glob != T - 1:
                    # skip the last add: n_{T-1} folds into the combine, so
                    # the tail never waits on GpSimd
                    nc.gpsimd.tensor_tensor(
                        out=nsum[:], in0=nsum[:], in1=n_cur[:],
                        op=mybir.AluOpType.add)
                t_glob += 1

        # combine: out = (1-d)*sum_{t<T-1} n + n_{T-1} + csum
        # (identity: (1-d)*sum_t n + d*n_{T-1} = (1-d)*N' + n_{T-1});
        # the host subtracts the constant d*kappa.
        n_last = sn[(T - 1) % 3]
        nc.vector.scalar_tensor_tensor(
            out=nsum[:], in0=nsum[:], scalar=1.0 - DECAY, in1=csum[:],
            op0=mybir.AluOpType.mult, op1=mybir.AluOpType.add)
        nc.vector.tensor_tensor(
            out=nsum[:], in0=n_last[:], in1=nsum[:],
            op=mybir.AluOpType.add)

        nc.sync.dma_start(o_d[:], nsum[:])

    nc.finalize()
    return nc


def _prep_x(x_core: np.ndarray) -> np.ndarray:
    """x_core [T, BL, C] -> fp16 [128, cols] block-major (k, m) layout, with
    sum_t x appended as 16 extra m-columns per k-slice of block XBLK."""
    xm = np.ascontiguousarray(x_core.reshape(T * BL, C).T)   # [C, M] = [k*128+p, m]
    xk = xm.reshape(NK, 128, T * BL)                         # [k, p, m]
    xs = x_core.sum(axis=0, dtype=np.float64).T.astype(np.float32)  # [C, BL]
    xsk = xs.reshape(NK, 128, BL)                            # [k, p, b]
    segs = []
    m0 = 0
    for bi, tb in enumerate(TBLOCKS):
        S = BL * tb
        seg = xk[:, :, m0:m0 + S]                            # [k, p, S]
        if bi == XBLK:
            seg = np.concatenate([seg, xsk], axis=2)         # [k, p, S+16]
        segs.append(np.ascontiguousarray(seg.transpose(1, 0, 2)).reshape(128, -1))
        m0 += S
    return np.concatenate(segs, axis=1).astype(np.float16)


def kernel(x: np.ndarray, W: np.ndarray, b: np.ndarray) -> np.ndarray:
    x = np.asarray(x, dtype=np.float32)
    W = np.asarray(W, dtype=np.float32)
    b = np.asarray(b, dtype=np.float32)
    use_bias = bool(np.any(b != 0.0))

    key = use_bias
    if key not in _cache:
        _cache[key] = _build(use_bias)
    nc = _cache[key]

    Wp = np.zeros((C, OP), np.float32)
    Wp[:, :O] = W
    # j-major chunks: [128, (j, k, 128)]
    wprep = np.ascontiguousarray(
        Wp.reshape(NK, 128, NJ, 128).transpose(1, 2, 0, 3)
          .reshape(128, NJ * NK * 128)).astype(np.float16)

    in_maps = []
    for c in range(NCORES):
        m = {"xp": _prep_x(x[:, c * BL:(c + 1) * BL, :]), "wp": wprep}
        if use_bias:
            bp = np.zeros((1, OP), np.float16)
            bp[0, :O] = b.astype(np.float16)
            m["bp"] = bp
        in_maps.append(m)

    res = run_bass_kernel_spmd(nc, in_maps, list(range(NCORES)))

    outs = []
    for c in range(NCORES):
        raw = res.results[c]["acc_raw"]                      # [o_p, (j, b)]
        raw = raw - np.float32(DECAY * KAPPA)                # device skips -d*kappa
        rate = raw.reshape(128, NJ, BL).transpose(2, 1, 0).reshape(BL, OP)
        outs.append(rate[:, :O] / np.float32(T))
    return np.concatenate(outs, axis=0).astype(np.float32)
